# revision 5
# baseline (speedup 1.0000x reference)
"""Trainium2 Bass kernel for DigitConvolutionalModel forward pass.

Model: x[B,784] -> 3x3 valid conv (28x28 -> 26x26) -> flatten[676]
       -> Linear(676->200) + ReLU -> Linear(200->10).

Key algebraic optimization: the conv is linear and feeds straight into the
first Linear, so both fold into a single effective weight
W_eff[200,784] = w0 compose conv  (computed once on host, ~1.2 MFLOP).
The device then runs two dense GEMMs per batch shard:
    h = relu(x @ W_eff.T + b0);  out = h @ w1.T + b1

Sharding: pure data parallel over the batch dim across 8 NeuronCores
(4096 rows each); weights replicated; no collectives (forward only).

On-device layout is feature-major ("transposed") so the contraction dim
always lives on SBUF partitions: xT[784,n] -> hT[200,n] -> outT[10,n].

DMA schedule. Measured DMA cost is ~30 ns per PARTITION LINE per queue
(byte rate only matters once a line exceeds ~10 KB), and the three
usable queues (SP / ACT / SWDGE) have independent line pumps. So:
 - the entire startup set (w0, w1, biases, first 384 batch cols of x)
   is packed by the host into ONE wide bf16 "urgent" image [128, 4111]
   whose 128 rows are split across the three queues (43 lines each,
   ~1.3 us) -> everything the first ~20 matmul groups need lands ~2.7 us
   after the engines start issuing.
 - the rest of x moves as wide multi-chunk images (up to 1024 batch
   cols = 14 KB lines, the byte-bound regime), again row-split three
   ways, in consumption order, depth-2 per queue.
 - mid-stream output stores queue up on SWDGE behind its ladder (they
   are latency-tolerant; every chunk has its own obuf slot); the last
   store rides SP for a short postamble fence.
Micro-warmup matmuls (64 rows) on zeroed scratch bridge the DMA flight
and trip the PE's HAM clock gate with fine granularity. Compute dtype
bf16 (1 cyc/row matmuls); PSUM accumulates f32; bias+ReLU fused on the
vector engine; layer-2 bias-add also on the vector engine so the two
HWDGE engines issue nothing but x DMAs.
"""

import os
import sys
import types
import numpy as np

for _p in ("/opt/trn_rl_repo", "/root/.axon_site"):
    if os.path.isdir(_p) and _p not in sys.path:
        sys.path.insert(0, _p)

import concourse.bass as bass  # noqa: E402
import concourse.tile as tile  # noqa: E402
import concourse.mybir as mybir  # noqa: E402
from concourse import bacc  # noqa: E402
from concourse.bass_utils import run_bass_kernel_spmd  # noqa: E402

B = 32768
N_CORES = 8
SHARD = B // N_CORES          # 4096
KDIM = 784                    # 28*28 input features (conv folded in)
HID = 200
OUT = 10
KT = 112                      # k-tile partition size (7 * 112 = 784)
NKT = KDIM // KT              # 7 k-tiles
M_TILES = [(0, 128), (128, 72)]  # hidden 200 = 128 + 72 PSUM partition tiles
N_WARMUP = 32                 # micro warmup matmuls (64 rows each)
WARM_ROWS = 64

# urgent-image column layout (bf16, 128 rows)
OFF_W0M0 = 0                      # NKT*128 cols, rows 0:112
OFF_W0M1 = OFF_W0M0 + NKT * 128   # NKT*72 cols, rows 0:112
OFF_W1 = OFF_W0M1 + NKT * 72      # 20 cols, rows 0:128
OFF_BIA = OFF_W1 + 2 * OUT        # 3 cols (b0[0:128] | b0[128:200] | b1)
OFF_X = OFF_BIA + 3               # first x chunks, rows 0:112
U_XCHUNKS = [128, 256]            # batch cols of x packed into the image
U_COLS = OFF_X + NKT * sum(U_XCHUNKS)

# bulk x images: (name, batch cols). 384 cols ride in the urgent image;
# chunks of <=512 batch cols are carved out of each image for the PE.
BULK = [("b1", 512), ("b2", 1024), ("b3", 1024), ("b4", 1024), ("b5", 128)]
assert sum(U_XCHUNKS) + sum(w for _, w in BULK) == SHARD

# per-queue row split of every image (3 queues with independent pumps)
def _rsplit(rows):
    t = (rows + 2) // 3
    return [(0, t), (t, 2 * t), (2 * t, rows)]

MM_DT = mybir.dt.bfloat16

last_exec_time_ns = None      # set when BASS_KERNEL_PROFILE=1


def _install_ntff_hook():
    """Register the axon NTFF profile hook if the image's antenv lacks it."""
    try:
        from antenv.axon_hooks import get_axon_ntff_profile_hook  # noqa: F401
        return
    except ImportError:
        pass
    try:
        from trn_agent_boot.trn_boot import _ntff_profile_via_ctypes
        hook = _ntff_profile_via_ctypes("/opt/axon/libaxon_pjrt.so")
    except Exception:
        hook = None
    mod = types.ModuleType("antenv.axon_hooks")
    mod.get_axon_ntff_profile_hook = lambda: hook
    mod.set_axon_ntff_profile_hook = lambda h: None
    sys.modules["antenv.axon_hooks"] = mod


def _np_mm_dtype():
    import ml_dtypes
    return np.dtype(ml_dtypes.bfloat16)


def fold_conv_into_fc(conv_w: np.ndarray, w0: np.ndarray) -> np.ndarray:
    """W_eff[200,784] such that x @ W_eff.T == fc1(flatten(conv(x)))."""
    w0v = w0.reshape(HID, 26, 26).astype(np.float64)
    w_img = np.zeros((HID, 28, 28), dtype=np.float64)
    for ki in range(3):
        for kj in range(3):
            w_img[:, ki:ki + 26, kj:kj + 26] += w0v * np.float64(conv_w[ki, kj])
    return w_img.reshape(HID, KDIM).astype(np.float32)


def _x_image(xsv, c0, w, mm_np):
    """[112, NKT*w] feature-major image of batch cols [c0, c0+w)."""
    blk = xsv[c0:c0 + w]                          # [n, a, p]
    return np.ascontiguousarray(
        blk.transpose(2, 1, 0).astype(mm_np)).reshape(KT, NKT * w)


def pack_inputs(x_shard, w_eff, w1, b0, b1, mm_np):
    """Build the urgent image + bulk x images for one core."""
    xsv = x_shard.reshape(SHARD, NKT, KT)
    u = np.zeros((128, U_COLS), dtype=mm_np)
    w0sb = w_eff.reshape(HID, NKT, KT).transpose(2, 1, 0)  # [p, a, m] f32
    u[0:KT, OFF_W0M0:OFF_W0M1] = w0sb[:, :, 0:128].reshape(KT, -1).astype(mm_np)
    u[0:KT, OFF_W0M1:OFF_W1] = w0sb[:, :, 128:HID].reshape(KT, -1).astype(mm_np)
    u[:, OFF_W1 + 0:OFF_W1 + OUT] = w1[:, 0:128].T.astype(mm_np)
    u[0:HID - 128, OFF_W1 + OUT:OFF_W1 + 2 * OUT] = w1[:, 128:HID].T.astype(mm_np)
    u[0:128, OFF_BIA] = b0[0:128].astype(mm_np)
    u[0:HID - 128, OFF_BIA + 1] = b0[128:HID].astype(mm_np)
    u[0:OUT, OFF_BIA + 2] = b1.astype(mm_np)
    c0 = 0
    off = OFF_X
    for w in U_XCHUNKS:
        u[0:KT, off:off + NKT * w] = _x_image(xsv, c0, w, mm_np)
        c0 += w
        off += NKT * w
    arrays = {}
    for j, (r0, r1) in enumerate(_rsplit(128)):
        arrays[f"u_{j}"] = np.ascontiguousarray(u[r0:r1])
    for name, w in BULK:
        img = _x_image(xsv, c0, w, mm_np)
        c0 += w
        for j, (r0, r1) in enumerate(_rsplit(KT)):
            arrays[f"{name}_{j}"] = np.ascontiguousarray(img[r0:r1])
    return arrays


def build_program():
    nc = bacc.Bacc("TRN2", target_bir_lowering=False, debug=False)
    f32 = mybir.dt.float32
    add = mybir.AluOpType.add
    amax = mybir.AluOpType.max

    u_d = [nc.declare_dram_parameter(f"u_{j}", [r1 - r0, U_COLS], MM_DT,
                                     isOutput=False)
           for j, (r0, r1) in enumerate(_rsplit(128))]
    bulk_d = {}
    for name, w in BULK:
        for j, (r0, r1) in enumerate(_rsplit(KT)):
            bulk_d[(name, j)] = nc.declare_dram_parameter(
                f"{name}_{j}", [r1 - r0, NKT * w], MM_DT, isOutput=False)
    out_d = nc.declare_dram_parameter("out", [OUT, SHARD], f32, isOutput=True)

    with tile.TileContext(nc) as tc:
        with (
            tc.tile_pool(name="weights", bufs=1) as wpool,
            tc.tile_pool(name="xin", bufs=1) as xpool,
            tc.tile_pool(name="hbuf", bufs=2) as hpool,
            tc.tile_pool(name="obuf", bufs=12) as opool,
            tc.tile_pool(name="psum", bufs=2, space=bass.MemorySpace.PSUM) as pp,
            tc.tile_pool(name="opsum", bufs=2, space=bass.MemorySpace.PSUM) as op,
        ):
            queues = [nc.sync, nc.scalar, nc.gpsimd]
            chains = {id(q): [] for q in queues}

            def chained_dma(eng, dst_ap, src_ap):
                dma = eng.dma_start(dst_ap, src_ap)
                chain = chains[id(eng)]
                if len(chain) >= 2:
                    # depth 2: ring busy across completion->issue gaps, but
                    # later transfers can't round-robin-steal much from the
                    # one the PE needs next
                    tile.add_dep_helper(
                        dma.ins, chain[-2].ins, sync=True,
                        reason="cap queue depth at 2")
                chain.append(dma)
                return dma

            # PE warmup scratch first so warmups start immediately
            warm_x = wpool.tile([KT, 128], MM_DT)
            nc.gpsimd.memset(warm_x[:], 0.0)

            # urgent image: rows split across the three queues
            U = wpool.tile([128, U_COLS], MM_DT, name="urgent")
            for j, (r0, r1) in enumerate(_rsplit(128)):
                chained_dma(queues[j], U[r0:r1, :], u_d[j][:])

            # bulk x images, row-split three ways, in consumption order
            bulk_t = {}
            for name, w in BULK:
                bt = xpool.tile([KT, NKT * w], MM_DT, tag=name, name=name)
                bulk_t[name] = bt
                for j, (r0, r1) in enumerate(_rsplit(KT)):
                    chained_dma(queues[j], bt[r0:r1, :], bulk_d[(name, j)][:])

            # PE pre-warm on zeroed scratch while the urgent image flies
            warm_ps = op.tile([128, WARM_ROWS], f32, tag="warm", bufs=1)
            for _ in range(N_WARMUP):
                nc.tensor.matmul(
                    warm_ps[:], warm_x[:, 0:128], warm_x[:, 0:WARM_ROWS],
                    start=True, stop=True)

            # biases to f32 once (DVE converts dtypes on copy)
            bia = wpool.tile([128, 3], f32)
            nc.vector.tensor_scalar_add(bia[:], U[0:128, OFF_BIA:OFF_BIA + 3], 0.0)

            def w0_ap(mi, a):
                if mi == 0:
                    o = OFF_W0M0 + a * 128
                    return U[0:KT, o:o + 128]
                o = OFF_W0M1 + a * 72
                return U[0:KT, o:o + 72]

            # chunk list: (width, rhs tile, col offset within image)
            chunks = []
            off = OFF_X
            for w in U_XCHUNKS:
                chunks.append((w, U, off))
                off += NKT * w
            for name, w in BULK:
                o = 0
                while o < w:
                    cw = min(512, w - o)
                    chunks.append((cw, bulk_t[name], o))
                    o += cw

            def rhs_ap(tile_, img_w, coff, a, w):
                # image layout [rows, a*img_w + n]: k-tile a, cols coff..+w
                o = a * img_w + coff
                return tile_[0:KT, o:o + w]

            def emit_layer2(g, w, c0, h_tiles):
                o_ps = op.tile([OUT, w], f32, tag="ops", name=f"ops_{g}")
                nc.tensor.matmul(
                    o_ps[:], U[0:128, OFF_W1:OFF_W1 + OUT], h_tiles[0][:],
                    start=True, stop=False)
                nc.tensor.matmul(
                    o_ps[:], U[0:HID - 128, OFF_W1 + OUT:OFF_W1 + 2 * OUT],
                    h_tiles[1][:], start=False, stop=True)
                o_sb = opool.tile([OUT, w], f32, tag="osb", name=f"osb_{g}")
                nc.vector.tensor_scalar_add(o_sb[:], o_ps[:], bia[0:OUT, 2:3])
                if g == len(chunks) - 1:
                    nc.sync.dma_start(out_d[:, c0:c0 + w], o_sb[:])
                else:
                    dma = nc.gpsimd.dma_start(out_d[:, c0:c0 + w], o_sb[:])
                    chains[id(nc.gpsimd)].append(dma)

            c0 = 0
            pending = None   # layer 2 runs one chunk behind layer 1
            for g, (w, tile_, coff) in enumerate(chunks):
                img_w = (tile_.shape[1] if tile_ is not U else None)
                h_tiles = []
                for mi, (m0, dm) in enumerate(M_TILES):
                    h_ps = pp.tile([dm, w], f32, tag=f"hps{mi}",
                                   name=f"hps_{g}_{mi}")
                    for a in range(NKT):
                        if tile_ is U:
                            # urgent image: x chunk cols start at coff with
                            # per-chunk contiguous [a][n] layout
                            o = coff + a * w
                            rhs = U[0:KT, o:o + w]
                        else:
                            iw = tile_.shape[1] // NKT
                            o = a * iw + coff
                            rhs = tile_[0:KT, o:o + w]
                        nc.tensor.matmul(
                            h_ps[:], w0_ap(mi, a), rhs,
                            start=(a == 0), stop=(a == NKT - 1))
                    h_sb = hpool.tile([dm, w], MM_DT, tag=f"h{mi}",
                                      name=f"h_{g}_{mi}")
                    nc.vector.tensor_scalar(
                        h_sb[:], h_ps[:], bia[0:dm, mi:mi + 1], 0.0,
                        add, amax)
                    h_tiles.append(h_sb)

                if pending is not None:
                    emit_layer2(*pending)
                pending = (g, w, c0, h_tiles)
                c0 += w

            emit_layer2(*pending)

    nc.compile()
    return nc


_program_cache = {}


def _get_program():
    key = (tuple(U_XCHUNKS), tuple(BULK), N_WARMUP)
    if key not in _program_cache:
        _program_cache[key] = build_program()
    return _program_cache[key]


def kernel(**inputs: np.ndarray) -> np.ndarray:
    x = np.asarray(inputs["x"], dtype=np.float32)
    conv_w = np.asarray(inputs["conv_w"], dtype=np.float32)
    w0 = np.asarray(inputs["w0"], dtype=np.float32)
    b0 = np.asarray(inputs["b0"], dtype=np.float32)
    w1 = np.asarray(inputs["w1"], dtype=np.float32)
    b1 = np.asarray(inputs["b1"], dtype=np.float32)

    mm_np = _np_mm_dtype()
    w_eff = fold_conv_into_fc(conv_w, w0)

    in_maps = []
    for i in range(N_CORES):
        in_maps.append(pack_inputs(
            x[i * SHARD:(i + 1) * SHARD], w_eff, w1, b0, b1, mm_np))

    nc = _get_program()

    profile = os.environ.get("BASS_KERNEL_PROFILE", "0") == "1"
    kwargs = {}
    if profile:
        _install_ntff_hook()
        kwargs = dict(trace=True, tmpdir=os.environ.get("BASS_KERNEL_TRACE_DIR"))
    try:
        res = run_bass_kernel_spmd(
            nc, in_maps, core_ids=list(range(N_CORES)), **kwargs)
    except Exception:
        # a previous process can leave a NeuronCore momentarily
        # unrecoverable (NRT_EXEC_UNIT_UNRECOVERABLE); one retry suffices
        import time
        time.sleep(5)
        res = run_bass_kernel_spmd(
            nc, in_maps, core_ids=list(range(N_CORES)), **kwargs)

    global last_exec_time_ns
    last_exec_time_ns = res.exec_time_ns

    out = np.empty((B, OUT), dtype=np.float32)
    for i in range(N_CORES):
        out[i * SHARD:(i + 1) * SHARD] = res.results[i]["out"].T
    return out


# revision 6
# speedup vs baseline: 2.5597x; 2.5597x over previous
"""Trainium2 Bass kernel for DigitConvolutionalModel forward pass.

Model: x[B,784] -> 3x3 valid conv (28x28 -> 26x26) -> flatten[676]
       -> Linear(676->200) + ReLU -> Linear(200->10).

Key algebraic optimization: the conv is linear and feeds straight into the
first Linear, so both fold into a single effective weight
W_eff[200,784] = w0 compose conv  (computed once on host, ~1.2 MFLOP).
The device then runs two dense GEMMs per batch shard:
    h = relu(x @ W_eff.T + b0);  out = h @ w1.T + b1

Sharding: pure data parallel over the batch dim across 8 NeuronCores
(4096 rows each); weights replicated; no collectives (forward only).

On-device layout is feature-major ("transposed") so the contraction dim
always lives on SBUF partitions: xT[784,n] -> hT[200,n] -> outT[10,n].

DMA facts this schedule is built around (measured): a transfer only
reaches queue rate (~210-235 GB/s) when it spans all ~112 partitions as
one whole-tile transfer with multi-KB lines; each queue round-robins
across its in-flight transfers; the three queues (SP / ACT / SWDGE)
stream independently. So:
 - x moves as whole per-segment images [112, NKT*w]; segments are
   small at the head (the first 64-col image lands ~1.5 us after issue
   and gates the first real matmul) and 512 wide in steady state.
 - the whole shard stays resident in SBUF (xin bufs = n segments) so no
   x DMA is ever gated on compute progress; explicit DMA-to-DMA deps
   cap each queue's in-flight depth at 2 (depth 1 for the latency-
   critical first hops so nothing round-robin-steals from them).
 - w0's halves ride ACT interleaved with the early odd segments; bias,
   w1 and one mid segment ride SWDGE; output stores queue on SWDGE
   (latency-tolerant, each chunk has its own obuf slot); the last store
   rides SP for a short postamble fence.
Micro-warmup matmuls (64 rows) on zeroed scratch bridge the DMA flight
and trip the PE's HAM clock gate with fine granularity. Compute dtype
bf16 (1 cyc/row matmuls); PSUM accumulates f32; bias+ReLU fused on the
vector engine; layer-2 bias-add also on the vector engine so the HWDGE
engines issue nothing but DMAs.
"""

import os
import sys
import types
import numpy as np

for _p in ("/opt/trn_rl_repo", "/root/.axon_site"):
    if os.path.isdir(_p) and _p not in sys.path:
        sys.path.insert(0, _p)

import concourse.bass as bass  # noqa: E402
import concourse.tile as tile  # noqa: E402
import concourse.mybir as mybir  # noqa: E402
from concourse import bacc  # noqa: E402
from concourse.bass_utils import run_bass_kernel_spmd  # noqa: E402

B = 32768
N_CORES = 8
SHARD = B // N_CORES          # 4096
KDIM = 784                    # 28*28 input features (conv folded in)
HID = 200
OUT = 10
SEGS = [64, 128, 256, 512, 512, 512, 512, 512, 512, 512, 64]
KT = 112                      # k-tile partition size (7 * 112 = 784)
NKT = KDIM // KT              # 7 k-tiles
M_TILES = [(0, 128), (128, 72)]  # hidden 200 = 128 + 72 PSUM partition tiles
N_WARMUP = 28                 # micro warmup matmuls (64 rows each)
WARM_ROWS = 64

MM_DT = mybir.dt.bfloat16

last_exec_time_ns = None      # set when BASS_KERNEL_PROFILE=1


def _install_ntff_hook():
    """Register the axon NTFF profile hook if the image's antenv lacks it."""
    try:
        from antenv.axon_hooks import get_axon_ntff_profile_hook  # noqa: F401
        return
    except ImportError:
        pass
    try:
        from trn_agent_boot.trn_boot import _ntff_profile_via_ctypes
        hook = _ntff_profile_via_ctypes("/opt/axon/libaxon_pjrt.so")
    except Exception:
        hook = None
    mod = types.ModuleType("antenv.axon_hooks")
    mod.get_axon_ntff_profile_hook = lambda: hook
    mod.set_axon_ntff_profile_hook = lambda h: None
    sys.modules["antenv.axon_hooks"] = mod


def _np_mm_dtype():
    import ml_dtypes
    return np.dtype(ml_dtypes.bfloat16)


def fold_conv_into_fc(conv_w: np.ndarray, w0: np.ndarray) -> np.ndarray:
    """W_eff[200,784] such that x @ W_eff.T == fc1(flatten(conv(x)))."""
    w0v = w0.reshape(HID, 26, 26).astype(np.float64)
    w_img = np.zeros((HID, 28, 28), dtype=np.float64)
    for ki in range(3):
        for kj in range(3):
            w_img[:, ki:ki + 26, kj:kj + 26] += w0v * np.float64(conv_w[ki, kj])
    return w_img.reshape(HID, KDIM).astype(np.float32)


def pack_shard(xs: np.ndarray, mm_np):
    """Pack one x shard [4096, 784] into per-segment SBUF tile images.

    Segment g (width w starting at col c0):
      xg[p, a, n] = x[c0 + n, a*KT + p]
    Every SBUF partition line is one contiguous (a, n) run.
    """
    xsv = xs.reshape(SHARD, NKT, KT)
    arrays = {}
    c0 = 0
    for g, w in enumerate(SEGS):
        blk = xsv[c0:c0 + w]                        # [n, a, p]
        arrays[f"xg{g}"] = np.ascontiguousarray(
            blk.transpose(2, 1, 0).astype(mm_np))   # [p, a, n]
        c0 += w
    return arrays


def pack_weights(w_eff: np.ndarray, w1: np.ndarray, b0, b1, mm_np):
    """Pack weights/biases into single-DMA SBUF images."""
    w0sb = np.ascontiguousarray(
        w_eff.reshape(HID, NKT, KT).transpose(2, 1, 0).astype(mm_np))
    w0a = np.ascontiguousarray(w0sb[:, :, 0:128])
    w0b = np.ascontiguousarray(w0sb[:, :, 128:HID])
    w1sb = np.zeros((128, 2 * OUT), dtype=mm_np)
    w1sb[:, :OUT] = w1[:, 0:128].T.astype(mm_np)
    w1sb[:HID - 128, OUT:] = w1[:, 128:HID].T.astype(mm_np)
    biases = np.zeros((128, 3), dtype=np.float32)
    biases[:, 0] = b0[0:128]
    biases[:HID - 128, 1] = b0[128:HID]
    biases[:OUT, 2] = b1
    return w0a, w0b, w1sb, biases


def build_program():
    nc = bacc.Bacc("TRN2", target_bir_lowering=False, debug=False)
    f32 = mybir.dt.float32
    add = mybir.AluOpType.add
    amax = mybir.AluOpType.max

    xg_d = [
        nc.declare_dram_parameter(
            f"xg{g}", [KT, NKT, w], MM_DT, isOutput=False)
        for g, w in enumerate(SEGS)
    ]
    w0a_d = nc.declare_dram_parameter("w0a", [KT, NKT, 128], MM_DT, isOutput=False)
    w0b_d = nc.declare_dram_parameter("w0b", [KT, NKT, HID - 128], MM_DT, isOutput=False)
    w1_d = nc.declare_dram_parameter("w1sb", [128, 2 * OUT], MM_DT, isOutput=False)
    bia_d = nc.declare_dram_parameter("biases", [128, 3], f32, isOutput=False)
    out_d = nc.declare_dram_parameter("out", [OUT, SHARD], f32, isOutput=True)

    with tile.TileContext(nc) as tc:
        with (
            tc.tile_pool(name="weights", bufs=1) as wpool,
            # whole shard resident: no x DMA is ever gated on compute
            tc.tile_pool(name="xin", bufs=len(SEGS)) as xpool,
            tc.tile_pool(name="hbuf", bufs=2) as hpool,
            tc.tile_pool(name="obuf", bufs=len(SEGS) + 1) as opool,
            tc.tile_pool(name="psum", bufs=2, space=bass.MemorySpace.PSUM) as pp,
            tc.tile_pool(name="opsum", bufs=2, space=bass.MemorySpace.PSUM) as op,
        ):
            # tiles
            w0t = [wpool.tile([KT, NKT, dm], MM_DT, name=f"w0_{mi}")
                   for mi, (m0, dm) in enumerate(M_TILES)]
            bia = wpool.tile([128, 3], f32)
            w1 = wpool.tile([128, 2 * OUT], MM_DT)
            xg_t = [xpool.tile([KT, NKT, w], MM_DT, tag="xg", name=f"xg_{g}")
                    for g, w in enumerate(SEGS)]

            warm_x = wpool.tile([KT, 128], MM_DT)
            nc.gpsimd.memset(warm_x[:], 0.0)

            # DMA schedule: (engine, dst, src, dep-index-in-ring or None)
            # sync:   s0 | s2 s4 s8 s10    (s2 waits s0 alone: depth 1)
            # scalar: w0a s1 | w0b s3 s5 s7 s9
            # gpsimd: bias w1 s6 | stores...
            sched = [
                (nc.sync, xg_t[0][:], xg_d[0][:], None),
                (nc.scalar, w0t[0][:], w0a_d[:], None),
                (nc.scalar, xg_t[1][:], xg_d[1][:], None),
                (nc.gpsimd, bia[:], bia_d[:], None),
                (nc.gpsimd, w1[:], w1_d[:], None),
                (nc.sync, xg_t[2][:], xg_d[2][:], 0),      # waits s0
                (nc.scalar, w0t[1][:], w0b_d[:], 0),       # waits w0a
                (nc.gpsimd, xg_t[6][:], xg_d[6][:], None),
                (nc.sync, xg_t[3][:], xg_d[3][:], 0),
                (nc.scalar, xg_t[4][:], xg_d[4][:], 1),
                (nc.sync, xg_t[5][:], xg_d[5][:], 1),
                (nc.scalar, xg_t[7][:], xg_d[7][:], 2),
                (nc.sync, xg_t[8][:], xg_d[8][:], 2),
                (nc.scalar, xg_t[9][:], xg_d[9][:], 3),
                (nc.sync, xg_t[10][:], xg_d[10][:], 3),
            ]
            rings = {id(nc.sync): [], id(nc.scalar): [], id(nc.gpsimd): []}
            for eng, dst, src, dep in sched:
                dma = eng.dma_start(dst, src)
                ring = rings[id(eng)]
                if dep is not None:
                    tile.add_dep_helper(
                        dma.ins, ring[dep].ins, sync=True,
                        reason="ring depth cap")
                ring.append(dma)

            # PE pre-warm on zeroed scratch while the first DMAs fly
            warm_ps = op.tile([128, WARM_ROWS], f32, tag="warm", bufs=1)
            for _ in range(N_WARMUP):
                nc.tensor.matmul(
                    warm_ps[:], warm_x[:, 0:128], warm_x[:, 0:WARM_ROWS],
                    start=True, stop=True)

            def emit_layer2(g, w, c0, h_tiles):
                o_ps = op.tile([OUT, w], f32, tag="ops", name=f"ops_{g}")
                nc.tensor.matmul(
                    o_ps[:], w1[0:128, 0:OUT], h_tiles[0][:],
                    start=True, stop=False)
                nc.tensor.matmul(
                    o_ps[:], w1[0:HID - 128, OUT:2 * OUT], h_tiles[1][:],
                    start=False, stop=True)
                o_sb = opool.tile([OUT, w], f32, tag="osb", name=f"osb_{g}")
                # bias-add on the vector engine (HWDGE engine queues must
                # stay pure DMA so a stalled transfer never delays layer 2)
                nc.vector.tensor_scalar_add(o_sb[:], o_ps[:], bia[0:OUT, 2:3])
                if g == len(SEGS) - 1:
                    nc.sync.dma_start(out_d[:, c0:c0 + w], o_sb[:])
                else:
                    nc.gpsimd.dma_start(out_d[:, c0:c0 + w], o_sb[:])

            c0 = 0
            pending = None   # layer 2 runs one segment behind layer 1
            for g, w in enumerate(SEGS):
                xg = xg_t[g]
                h_tiles = []
                for mi, (m0, dm) in enumerate(M_TILES):
                    h_ps = pp.tile([dm, w], f32, tag=f"hps{mi}",
                                   name=f"hps_{g}_{mi}")
                    for a in range(NKT):
                        nc.tensor.matmul(
                            h_ps[:],
                            w0t[mi][:, a, :],
                            xg[:, a, :],
                            start=(a == 0),
                            stop=(a == NKT - 1),
                        )
                    h_sb = hpool.tile([dm, w], MM_DT, tag=f"h{mi}",
                                      name=f"h_{g}_{mi}")
                    nc.vector.tensor_scalar(
                        h_sb[:], h_ps[:], bia[0:dm, mi:mi + 1], 0.0,
                        add, amax)
                    h_tiles.append(h_sb)

                if pending is not None:
                    emit_layer2(*pending)
                pending = (g, w, c0, h_tiles)
                c0 += w

            emit_layer2(*pending)

    nc.compile()
    return nc


_program_cache = {}


def _get_program():
    key = (MM_DT, tuple(SEGS), N_WARMUP)
    if key not in _program_cache:
        _program_cache[key] = build_program()
    return _program_cache[key]


def kernel(**inputs: np.ndarray) -> np.ndarray:
    x = np.asarray(inputs["x"], dtype=np.float32)
    conv_w = np.asarray(inputs["conv_w"], dtype=np.float32)
    w0 = np.asarray(inputs["w0"], dtype=np.float32)
    b0 = np.asarray(inputs["b0"], dtype=np.float32)
    w1 = np.asarray(inputs["w1"], dtype=np.float32)
    b1 = np.asarray(inputs["b1"], dtype=np.float32)

    mm_np = _np_mm_dtype()
    w_eff = fold_conv_into_fc(conv_w, w0)
    w0a, w0b, w1sb, biases = pack_weights(w_eff, w1, b0, b1, mm_np)

    in_maps = []
    for i in range(N_CORES):
        m = pack_shard(x[i * SHARD:(i + 1) * SHARD], mm_np)
        m.update({"w0a": w0a, "w0b": w0b, "w1sb": w1sb, "biases": biases})
        in_maps.append(m)

    nc = _get_program()

    profile = os.environ.get("BASS_KERNEL_PROFILE", "0") == "1"
    kwargs = {}
    if profile:
        _install_ntff_hook()
        kwargs = dict(trace=True, tmpdir=os.environ.get("BASS_KERNEL_TRACE_DIR"))
    try:
        res = run_bass_kernel_spmd(
            nc, in_maps, core_ids=list(range(N_CORES)), **kwargs)
    except Exception:
        # a previous process can leave a NeuronCore momentarily
        # unrecoverable (NRT_EXEC_UNIT_UNRECOVERABLE); one retry suffices
        import time
        time.sleep(5)
        res = run_bass_kernel_spmd(
            nc, in_maps, core_ids=list(range(N_CORES)), **kwargs)

    global last_exec_time_ns
    last_exec_time_ns = res.exec_time_ns

    out = np.empty((B, OUT), dtype=np.float32)
    for i in range(N_CORES):
        out[i * SHARD:(i + 1) * SHARD] = res.results[i]["out"].T
    return out
